# revision 5
# baseline (speedup 1.0000x reference)
"""MultiLevelAlignedRoIPooling Trainium2 kernel (v2).

Strategy
--------
Output[b, n, i, j, c] = sum_{a,b' in {0,1}} wy_a(i) wx_b'(j) feat[y_a(i), x_b'(j), c]
(7x7 aligned bilinear RoI pooling; the reference's 2x2 avg pool is algebraically
the 4-tap bilinear interpolation at each of the 7x7 sample points).

With the reference's box distribution every box lands on pyramid level 4, so all
gathers read feat0 only (verified on host; numpy fallback otherwise).

Sharding: 8 cores = 4 batches x 2 halves of the 256 boxes. Each core handles
128 boxes (one per SBUF partition):
  - Host packs feat0[b] into a row-pair table (fp16): row (y,x) holds
    [feat[y,x,:], feat[y+1,x,:]], so ONE 2KB dma_gather element fetches the
    full 2x2 bilinear block of a sample point.
  - 7 gathers (one per sample column j), each delivering g[box, i, xtap, ytap, C].
  - x-combine on the (otherwise idle) TensorEngine: per-partition weights wx0(j),
    wx1(j) are placed on the diagonal of a 128x128 stationary matrix, so
    PSUM[p, f] = wx0[p]*g[p, xtap=0, f] + wx1[p]*g[p, xtap=1, f] via two
    accumulating matmuls.  Done in half-chunks (4+3 sample rows) so two PSUM
    tiles (4 banks each) ping-pong.
  - y-combine straight out of PSUM: ScalarE mul (u = ly*t1) + DVE
    scalar_tensor_tensor (o = hy*t0 + u), per sample row i.
  - Results stream to DRAM as [box, (i*7+j)*C] fp16 rows (i-major).

Host prep computes gather indices (int16) + tap weights with numpy f32 math
that mirrors the reference op-for-op.
"""

import os

import numpy as np

B, N, C = 4, 256, 256
H = W = 128
OUT = 7
NS = OUT * OUT            # 49 sample points per box
BOX_PER_CORE = 128
NCORES = 8
NIDX = BOX_PER_CORE * NS  # 6272 gathers per core
WCOLS = NIDX // 16        # 392 wrapped index columns (56 per chunk)

_NC_CACHE = None


def _build_nc():
    """Build + compile the per-core Bass program (same program on all cores)."""
    global _NC_CACHE
    if _NC_CACHE is not None:
        return _NC_CACHE
    from contextlib import ExitStack

    import concourse.bass as bass
    import concourse.tile as tile
    from concourse import bacc, mybir

    fdt = mybir.dt.float16
    f32 = mybir.dt.float32
    i16 = mybir.dt.int16
    mult = mybir.AluOpType.mult
    add = mybir.AluOpType.add

    nq = int(os.environ.get("KERNEL_NQUEUES", "2"))
    scr = int(os.environ.get("KERNEL_DMA_SCRATCH", "16384"))
    nc = bacc.Bacc(
        "TRN2", target_bir_lowering=False, debug=False, num_devices=NCORES,
        num_swdge_queues=nq, dynamic_dma_scratch_size=scr,
    )
    # feat_pairs: row r = pixel (y, x) holding [feat[y,x,:], feat[y+1,x,:]]
    feat = nc.dram_tensor("feat", [H * W, 2 * C], fdt, kind="ExternalInput")
    idx = nc.dram_tensor("idx", [128, WCOLS], i16, kind="ExternalInput")
    # y-combine weights: [hy(i) | ly(i)], OUT columns each
    wts = nc.dram_tensor("wts", [128, 2 * OUT], f32, kind="ExternalInput")
    # x-combine diagonal stationaries: slot t=2*j+xtap holds diag(wx_xtap(:, j))
    wdiag = nc.dram_tensor("wdiag", [128, 14 * 128], fdt, kind="ExternalInput")
    # i-major output: column (i*OUT + j)*C + c
    out = nc.dram_tensor("out", [128, NS * C], fdt, kind="ExternalOutput")

    HY, LY = 0, OUT

    with tile.TileContext(nc) as tc, ExitStack() as ctx:
        meta = ctx.enter_context(tc.tile_pool(name="meta", bufs=1))
        gp = ctx.enter_context(tc.tile_pool(name="g", bufs=3))
        pp = ctx.enter_context(tc.psum_pool(name="p", bufs=8))
        up = ctx.enter_context(tc.tile_pool(name="u", bufs=4))
        op = ctx.enter_context(tc.tile_pool(name="o", bufs=3))

        idx_t = meta.tile([128, WCOLS], i16, name="idx_t")
        wts_t = meta.tile([128, 2 * OUT], f32, name="wts_t")
        wd_t = meta.tile([128, 14 * 128], fdt, name="wd_t")
        # chunk 0's indices first so the first gather launches ASAP
        nc.sync.dma_start(idx_t[:, 0:56], idx.ap()[:, 0:56])
        nc.sync.dma_start(wts_t[:], wts.ap()[:, :])
        nc.sync.dma_start(idx_t[:, 56:WCOLS], idx.ap()[:, 56:WCOLS])
        nc.sync.dma_start(wd_t[:], wdiag.ap()[:, :])

        # Gather source: one elem covers pixels (y,xb),(y,xb+1) with both
        # y/y+1 rows each (row-pair layout), elem_step = one pixel pair.
        feat_gap = bass.AP(feat, 0, [[2 * C, H * W - 1], [1, 4 * C]])

        for j in range(OUT):
            # g layout: [128, i(7), xtap(2), ytap(2), C] for sample column j
            g = gp.tile([128, OUT, 2, 2, C], fdt, tag="g", name=f"g_{j}")
            nc.gpsimd.dma_gather(
                g.rearrange("p i x y c -> p i (x y c)"),
                feat_gap,
                idx_t[:, j * 56 : (j + 1) * 56],
                num_idxs=OUT * 128,
                num_idxs_reg=OUT * 128,
                elem_size=4 * C,
                elem_step=2 * C,
                queue_num=j % nq,
            )
            o = op.tile([128, OUT, C], fdt, tag="o", name=f"o_{j}")
            for i in range(OUT):
                # x-combine into one PSUM bank:
                # p = diag(wx0_j) @ g[i, xtap=0] + diag(wx1_j) @ g[i, xtap=1]
                p = pp.tile([128, 2 * C], f32, tag="p", name=f"p_{j}_{i}")
                nc.tensor.matmul(
                    p[:], wd_t[:, (2 * j) * 128 : (2 * j + 1) * 128],
                    g[:, i, 0, :, :], start=True, stop=False,
                )
                nc.tensor.matmul(
                    p[:], wd_t[:, (2 * j + 1) * 128 : (2 * j + 2) * 128],
                    g[:, i, 1, :, :], start=False, stop=True,
                )
                # y-combine: o[i] = hy(i)*p[ytap=0] + ly(i)*p[ytap=1]
                u = up.tile([128, C], f32, tag="u", name=f"u_{j}_{i}")
                nc.scalar.mul(u[:], p[:, C:], wts_t[:, LY + i : LY + i + 1])
                nc.vector.scalar_tensor_tensor(
                    o[:, i, :], p[:, :C], wts_t[:, HY + i : HY + i + 1],
                    u[:], mult, add,
                )
            nc.sync.dma_start(
                bass.AP(out, j * C, [[NS * C, 128], [OUT * C, OUT], [1, C]]),
                o[:],
            )

    nc.compile()
    _NC_CACHE = nc
    return nc


def _host_tables(boxes):
    """Numpy f32 replica of the reference's index/weight math.

    Returns None if any box is assigned a level other than 4 (never happens
    with the reference's input distribution), else per-core gather tables.
    """
    f32 = np.float32
    b = boxes.astype(f32)
    box_h = b[..., 2] - b[..., 0]
    box_w = b[..., 3] - b[..., 1]
    area = np.sqrt(box_h * box_w)
    with np.errstate(divide="ignore", invalid="ignore"):
        lev = np.floor(np.log(area / f32(224.0)) / np.log(f32(2.0))) + f32(4.0)
    if not np.all(np.isfinite(lev)):
        return None
    levels = np.clip(lev.astype(np.int32), 4, 64)
    if not np.all(levels == 4):
        return None
    scale = np.exp2(levels.astype(f32))
    bs = b / scale[..., None]
    bh = (box_h / scale).astype(f32)
    bw = (box_w / scale).astype(f32)
    by = (bs[..., 0] - f32(0.5)).astype(f32)
    bx = (bs[..., 1] - f32(0.5)).astype(f32)
    offs = ((np.arange(OUT, dtype=f32) + f32(0.5)) / f32(OUT)).astype(f32)
    gy = (by[..., None] + offs * bh[..., None]).astype(f32)  # [B,N,7]
    gx = (bx[..., None] + offs * bw[..., None]).astype(f32)
    y0 = np.maximum(f32(0.0), np.floor(gy))
    x0 = np.maximum(f32(0.0), np.floor(gx))
    bnd = f32(H - 1)
    y_lo = np.minimum(y0, bnd).astype(np.int32)
    y_hi = np.minimum(y0 + f32(1.0), bnd).astype(np.int32)
    x_lo = np.minimum(x0, bnd).astype(np.int32)
    x_hi = np.minimum(x0 + f32(1.0), bnd).astype(np.int32)
    ly = (gy - y0).astype(f32)
    lx = (gx - x0).astype(f32)
    hy = (f32(1.0) - ly).astype(f32)
    hx = (f32(1.0) - lx).astype(f32)
    # 2-pixel gather base in x; remap x-tap weights onto (xb, xb+1)
    xb = np.minimum(x_lo, W - 2)
    wx0 = hx * (x_lo == xb) + lx * (x_hi == xb)
    wx1 = hx * (x_lo == xb + 1) + lx * (x_hi == xb + 1)
    return y_lo, y_hi, xb, hy, ly, wx0.astype(f32), wx1.astype(f32)


def _feat_pairs(feat0_b):
    """[H*W, 2*C] row-pair layout: row (y*W+x) = [feat[y,x,:], feat[y+1,x,:]]
    (last row duplicates y=127, matching the reference's boundary clamp)."""
    fp = np.empty((H, W, 2, C), dtype=np.float16)
    fp[:, :, 0] = feat0_b
    fp[:-1, :, 1] = feat0_b[1:]
    fp[-1, :, 1] = feat0_b[-1]
    return np.ascontiguousarray(fp.reshape(H * W, 2 * C))


def _percore_inputs(featp_by_batch, tables, core):
    y_lo, y_hi, xb, hy, ly, wx0, wx1 = tables
    bat, half = divmod(core, 2)
    sl = slice(half * BOX_PER_CORE, (half + 1) * BOX_PER_CORE)
    ylo = y_lo[bat, sl]  # [128, 7]
    xbs = xb[bat, sl]
    # flat pixel index of the 2x2 block base, [128 box, 7 i, 7 j]
    i0 = (ylo[:, :, None] * W + xbs[:, None, :]).astype(np.int32)

    # gather sequence: g = (j*7 + i)*128 + box  (j-major sample order)
    seq = np.transpose(i0, (2, 1, 0)).reshape(NIDX).astype(np.int16)
    wr = seq.reshape(WCOLS, 16).T  # [16, WCOLS]
    idx = np.tile(wr, (8, 1))      # replicate across the 8 gpsimd cores

    hys = hy[bat, sl]    # [128, 7] per sample-row i
    lys = ly[bat, sl]
    wts = np.concatenate([hys, lys], axis=1).astype(np.float32)

    # diag stationaries [128, 14, 128] fp16: slot 2*j+xtap = diag(wx_xtap(:, j))
    wd = np.zeros((128, 14, 128), dtype=np.float16)
    pidx = np.arange(128)
    wvals = np.empty((128, 14), dtype=np.float16)
    wvals[:, 0::2] = wx0[bat, sl]
    wvals[:, 1::2] = wx1[bat, sl]
    wd[pidx[:, None], np.arange(14)[None, :], pidx[:, None]] = wvals

    return {
        "feat": featp_by_batch[bat],
        "idx": np.ascontiguousarray(idx),
        "wts": np.ascontiguousarray(wts),
        "wdiag": np.ascontiguousarray(wd.reshape(128, 14 * 128)),
    }


def _reference_numpy(feats, boxes):
    """Generic fallback: straight numpy port of the reference (never used
    with the reference input distribution; kept for safety)."""
    f32 = np.float32
    L = len(feats)
    padded = np.zeros((B, L, H, W, C), dtype=f32)
    for i, f in enumerate(feats):
        padded[:, i, : f.shape[1], : f.shape[2], :] = f
    b = boxes.astype(f32)
    box_h = b[..., 2] - b[..., 0]
    box_w = b[..., 3] - b[..., 1]
    area = np.sqrt(box_h * box_w)
    lev = np.floor(np.log(area / f32(224.0)) / np.log(f32(2.0))) + f32(4.0)
    levels = np.clip(lev.astype(np.int32), 4, 64)
    scale = np.exp2(levels.astype(f32))
    bs = b / scale[..., None]
    bh = box_h / scale
    bw = box_w / scale
    yxhw = np.concatenate([bs[..., 0:2], bh[..., None], bw[..., None]], axis=-1)
    lvl = levels - 4
    strides = np.exp2(lvl.astype(f32))
    bnd_h = H / strides - f32(1.0)
    bnd_w = W / strides - f32(1.0)
    by = bnd_w[..., None]  # faithful swap from the reference
    bx = bnd_h[..., None]
    box_y = yxhw[..., 0] - f32(0.5)
    box_x = yxhw[..., 1] - f32(0.5)
    offs = (np.arange(OUT, dtype=f32) + f32(0.5)) / f32(OUT)
    gy = box_y[..., None] + offs * yxhw[..., 2:3]
    gx = box_x[..., None] + offs * yxhw[..., 3:4]
    y0 = np.maximum(f32(0.0), np.floor(gy))
    x0 = np.maximum(f32(0.0), np.floor(gx))
    y01 = np.stack([np.minimum(y0, by), np.minimum(y0 + 1, by)], axis=3).reshape(
        B, N, 2 * OUT
    )
    x01 = np.stack([np.minimum(x0, bx), np.minimum(x0 + 1, bx)], axis=3).reshape(
        B, N, 2 * OUT
    )
    yi = y01.astype(np.int32)
    xi = x01.astype(np.int32)
    bi = np.arange(B)[:, None, None, None]
    li = np.clip(lvl, 0, L - 1)[:, :, None, None]
    gathered = padded[bi, li, yi[:, :, :, None], xi[:, :, None, :]]
    ly = gy - y0
    lx = gx - x0
    hy = 1.0 - ly
    hx = 1.0 - lx
    ky = np.stack([hy, ly], axis=3).reshape(B, N, 2 * OUT, 1)
    kx = np.stack([hx, lx], axis=3).reshape(B, N, 1, 2 * OUT)
    kern = (ky * kx * 4.0).astype(f32)
    weighted = gathered * kern[..., None]
    out = weighted.reshape(B, N, OUT, 2, OUT, 2, C).mean(axis=(3, 5))
    return out.astype(f32)


_TRACE_TMPDIR = None


def _run(in_maps, trace=False):
    from concourse.bass_utils import run_bass_kernel_spmd

    nc = _build_nc()
    kw = {}
    if trace and _TRACE_TMPDIR:
        kw["tmpdir"] = _TRACE_TMPDIR
    return run_bass_kernel_spmd(nc, in_maps, list(range(NCORES)), trace=trace, **kw)


def _kernel_impl(inputs, trace=False):
    feats = [np.asarray(inputs[f"feat{i}"], dtype=np.float32) for i in range(5)]
    boxes = np.asarray(inputs["boxes"], dtype=np.float32)
    tables = _host_tables(boxes)
    if tables is None:
        return _reference_numpy(feats, boxes), None
    featp = [_feat_pairs(feats[0][b]) for b in range(B)]
    in_maps = [_percore_inputs(featp, tables, c) for c in range(NCORES)]
    res = _run(in_maps, trace=trace)
    full = np.empty((B, N, OUT, OUT, C), dtype=np.float32)
    for core in range(NCORES):
        bat, half = divmod(core, 2)
        # device sample order is (i, j) already
        o = res.results[core]["out"].astype(np.float32).reshape(
            BOX_PER_CORE, OUT, OUT, C
        )
        full[bat, half * BOX_PER_CORE : (half + 1) * BOX_PER_CORE] = o
    return full, res


def kernel(**inputs):
    out, _ = _kernel_impl(inputs)
    return out


def kernel_profiled(**inputs):
    """Like kernel() but with trace=True; returns (output, BassKernelResults)."""
    return _kernel_impl(inputs, trace=True)


# revision 8
# speedup vs baseline: 1.0061x; 1.0061x over previous
"""MultiLevelAlignedRoIPooling Trainium2 kernel (v2).

Strategy
--------
Output[b, n, i, j, c] = sum_{a,b' in {0,1}} wy_a(i) wx_b'(j) feat[y_a(i), x_b'(j), c]
(7x7 aligned bilinear RoI pooling; the reference's 2x2 avg pool is algebraically
the 4-tap bilinear interpolation at each of the 7x7 sample points).

With the reference's box distribution every box lands on pyramid level 4, so all
gathers read feat0 only (verified on host; numpy fallback otherwise).

Sharding: 8 cores = 4 batches x 2 halves of the 256 boxes. Each core handles
128 boxes (one per SBUF partition):
  - Host packs feat0[b] into a row-pair table (fp16): row (y,x) holds
    [feat[y,x,:], feat[y+1,x,:]], so ONE 2KB dma_gather element fetches the
    full 2x2 bilinear block of a sample point.
  - 7 gathers (one per sample column j), each delivering g[box, i, xtap, ytap, C].
  - x-combine on the (otherwise idle) TensorEngine: per-partition weights wx0(j),
    wx1(j) are placed on the diagonal of a 128x128 stationary matrix, so
    PSUM[p, f] = wx0[p]*g[p, xtap=0, f] + wx1[p]*g[p, xtap=1, f] via two
    accumulating matmuls.  Done in half-chunks (4+3 sample rows) so two PSUM
    tiles (4 banks each) ping-pong.
  - y-combine straight out of PSUM: ScalarE mul (u = ly*t1) + DVE
    scalar_tensor_tensor (o = hy*t0 + u), per sample row i.
  - Results stream to DRAM as [box, (i*7+j)*C] fp16 rows (i-major).

Host prep computes gather indices (int16) + tap weights with numpy f32 math
that mirrors the reference op-for-op.
"""

import os

import numpy as np

B, N, C = 4, 256, 256
H = W = 128
OUT = 7
NS = OUT * OUT            # 49 sample points per box
BOX_PER_CORE = 128
NCORES = 8
NIDX = BOX_PER_CORE * NS  # 6272 gathers per core
WCOLS = NIDX // 16        # 392 wrapped index columns (56 per chunk)

_NC_CACHE = None


def _build_nc():
    """Build + compile the per-core Bass program (same program on all cores)."""
    global _NC_CACHE
    if _NC_CACHE is not None:
        return _NC_CACHE
    from contextlib import ExitStack

    import concourse.bass as bass
    import concourse.tile as tile
    from concourse import bacc, mybir

    fdt = mybir.dt.float16
    f32 = mybir.dt.float32
    i16 = mybir.dt.int16
    mult = mybir.AluOpType.mult
    add = mybir.AluOpType.add

    nq = int(os.environ.get("KERNEL_NQUEUES", "2"))
    scr = int(os.environ.get("KERNEL_DMA_SCRATCH", "16384"))
    nc = bacc.Bacc(
        "TRN2", target_bir_lowering=False, debug=False, num_devices=NCORES,
        num_swdge_queues=nq, dynamic_dma_scratch_size=scr,
    )
    # feat_pairs: row r = pixel (y, x) holding [feat[y,x,:], feat[y+1,x,:]]
    feat = nc.dram_tensor("feat", [H * W, 2 * C], fdt, kind="ExternalInput")
    idx = nc.dram_tensor("idx", [128, WCOLS], i16, kind="ExternalInput")
    # y-combine weights: [hy(i) | ly(i)], OUT columns each
    wts = nc.dram_tensor("wts", [128, 2 * OUT], f32, kind="ExternalInput")
    # x-combine diagonal stationaries: slot t=2*j+xtap holds diag(wx_xtap(:, j))
    wdiag = nc.dram_tensor("wdiag", [128, 14 * 128], fdt, kind="ExternalInput")
    # i-major output: column (i*OUT + j)*C + c
    out = nc.dram_tensor("out", [128, NS * C], fdt, kind="ExternalOutput")

    HY, LY = 0, OUT

    HALVES = ((0, 4), (4, 7))
    JGROUPS = ((0, 3), (3, 5), (5, 7))

    with tile.TileContext(nc) as tc, ExitStack() as ctx:
        meta = ctx.enter_context(tc.tile_pool(name="meta", bufs=1))
        gp = ctx.enter_context(tc.tile_pool(name="g", bufs=3))
        ppA = ctx.enter_context(tc.psum_pool(name="pA", bufs=1))
        ppB = ctx.enter_context(tc.psum_pool(name="pB", bufs=1))
        up = ctx.enter_context(tc.tile_pool(name="u", bufs=4))
        op = ctx.enter_context(tc.tile_pool(name="o", bufs=2))

        idx_t = meta.tile([128, WCOLS], i16, name="idx_t")
        wts_t = meta.tile([128, 2 * OUT], f32, name="wts_t")
        wd_t = meta.tile([128, 14 * 128], fdt, name="wd_t")
        dummy_i = meta.tile([128, 8], i16, name="dummy_i")
        dummy_g = meta.tile([128, 1, 4 * C], fdt, name="dummy_g")
        # x-combined intermediate, i-major: t[box, i, j, ytap, C]
        t_t = meta.tile([128, OUT, OUT, 2, C], fdt, name="t_t")

        # chunk 0's indices first so the first gather launches ASAP
        nc.sync.dma_start(idx_t[:, 0:56], idx.ap()[:, 0:56])
        nc.sync.dma_start(wts_t[:], wts.ap()[:, :])
        nc.sync.dma_start(idx_t[:, 56:WCOLS], idx.ap()[:, 56:WCOLS])
        nc.sync.dma_start(wd_t[:], wdiag.ap()[:, :])

        # Gather source: one elem covers pixels (y,xb),(y,xb+1) with both
        # y/y+1 rows each (row-pair layout), elem_step = one pixel pair.
        feat_gap = bass.AP(feat, 0, [[2 * C, H * W - 1], [1, 4 * C]])

        # dummy warm-up gathers (no DMA dependency): absorb the SWDGE
        # startup latency while the idx table is still loading.
        nc.gpsimd.memset(dummy_i[:], 0)
        for q in range(nq):
            nc.gpsimd.dma_gather(
                dummy_g[:], feat_gap, dummy_i[:],
                num_idxs=128, num_idxs_reg=128,
                elem_size=4 * C, elem_step=2 * C, queue_num=q,
            )

        for j in range(OUT):
            # g layout: [128, i(7), xtap(2), ytap(2), C] for sample column j
            g = gp.tile([128, OUT, 2, 2, C], fdt, tag="g", name=f"g_{j}")
            nc.gpsimd.dma_gather(
                g.rearrange("p i x y c -> p i (x y c)"),
                feat_gap,
                idx_t[:, j * 56 : (j + 1) * 56],
                num_idxs=OUT * 128,
                num_idxs_reg=OUT * 128,
                elem_size=4 * C,
                elem_step=2 * C,
                queue_num=j % nq,
            )
            for hi, (ilo, ihi) in enumerate(HALVES):
                w = ihi - ilo
                pool = ppA if hi == 0 else ppB
                p = pool.tile([128, w * 2 * C], f32, tag=f"p{hi}", name=f"p_{j}_{hi}")
                for il in range(w):
                    # x-combine into one PSUM bank per i:
                    # p[il] = diag(wx0_j) @ g[i, 0] + diag(wx1_j) @ g[i, 1]
                    i = ilo + il
                    pb = p[:, il * 2 * C : (il + 1) * 2 * C]
                    nc.tensor.matmul(
                        pb, wd_t[:, (2 * j) * 128 : (2 * j + 1) * 128],
                        g[:, i, 0, :, :], start=True, stop=False,
                    )
                    nc.tensor.matmul(
                        pb, wd_t[:, (2 * j + 1) * 128 : (2 * j + 2) * 128],
                        g[:, i, 1, :, :], start=False, stop=True,
                    )
                # bulk PSUM -> SBUF (fp32 -> fp16), i-major placement
                dst = t_t[:, ilo:ihi, j, :, :]
                src = p.rearrange("p (i y c) -> p i y c", i=w, y=2)
                if hi == 0:
                    nc.vector.tensor_copy(dst, src)
                else:
                    nc.scalar.copy(dst, src)

            for glo, ghi in JGROUPS:
                if j != ghi - 1:
                    continue
                wg = ghi - glo
                # y-combine, wide over this j-group: o = hy(i)*t0 + ly(i)*t1
                og = op.tile([128, OUT, wg * C], fdt, tag="og", name=f"og_{glo}")
                for i in range(OUT):
                    u = up.tile([128, wg * C], fdt, tag="u", name=f"u_{glo}_{i}")
                    nc.scalar.mul(
                        u[:], t_t[:, i, glo:ghi, 1, :], wts_t[:, LY + i : LY + i + 1]
                    )
                    nc.vector.scalar_tensor_tensor(
                        og[:, i, :], t_t[:, i, glo:ghi, 0, :],
                        wts_t[:, HY + i : HY + i + 1], u[:], mult, add,
                    )
                nc.sync.dma_start(
                    bass.AP(out, glo * C,
                            [[NS * C, 128], [OUT * C, OUT], [1, wg * C]]),
                    og[:],
                )

    nc.compile()
    _NC_CACHE = nc
    return nc


def _host_tables(boxes):
    """Numpy f32 replica of the reference's index/weight math.

    Returns None if any box is assigned a level other than 4 (never happens
    with the reference's input distribution), else per-core gather tables.
    """
    f32 = np.float32
    b = boxes.astype(f32)
    box_h = b[..., 2] - b[..., 0]
    box_w = b[..., 3] - b[..., 1]
    area = np.sqrt(box_h * box_w)
    with np.errstate(divide="ignore", invalid="ignore"):
        lev = np.floor(np.log(area / f32(224.0)) / np.log(f32(2.0))) + f32(4.0)
    if not np.all(np.isfinite(lev)):
        return None
    levels = np.clip(lev.astype(np.int32), 4, 64)
    if not np.all(levels == 4):
        return None
    scale = np.exp2(levels.astype(f32))
    bs = b / scale[..., None]
    bh = (box_h / scale).astype(f32)
    bw = (box_w / scale).astype(f32)
    by = (bs[..., 0] - f32(0.5)).astype(f32)
    bx = (bs[..., 1] - f32(0.5)).astype(f32)
    offs = ((np.arange(OUT, dtype=f32) + f32(0.5)) / f32(OUT)).astype(f32)
    gy = (by[..., None] + offs * bh[..., None]).astype(f32)  # [B,N,7]
    gx = (bx[..., None] + offs * bw[..., None]).astype(f32)
    y0 = np.maximum(f32(0.0), np.floor(gy))
    x0 = np.maximum(f32(0.0), np.floor(gx))
    bnd = f32(H - 1)
    y_lo = np.minimum(y0, bnd).astype(np.int32)
    y_hi = np.minimum(y0 + f32(1.0), bnd).astype(np.int32)
    x_lo = np.minimum(x0, bnd).astype(np.int32)
    x_hi = np.minimum(x0 + f32(1.0), bnd).astype(np.int32)
    ly = (gy - y0).astype(f32)
    lx = (gx - x0).astype(f32)
    hy = (f32(1.0) - ly).astype(f32)
    hx = (f32(1.0) - lx).astype(f32)
    # 2-pixel gather base in x; remap x-tap weights onto (xb, xb+1)
    xb = np.minimum(x_lo, W - 2)
    wx0 = hx * (x_lo == xb) + lx * (x_hi == xb)
    wx1 = hx * (x_lo == xb + 1) + lx * (x_hi == xb + 1)
    return y_lo, y_hi, xb, hy, ly, wx0.astype(f32), wx1.astype(f32)


def _feat_pairs(feat0_b):
    """[H*W, 2*C] row-pair layout: row (y*W+x) = [feat[y,x,:], feat[y+1,x,:]]
    (last row duplicates y=127, matching the reference's boundary clamp)."""
    fp = np.empty((H, W, 2, C), dtype=np.float16)
    fp[:, :, 0] = feat0_b
    fp[:-1, :, 1] = feat0_b[1:]
    fp[-1, :, 1] = feat0_b[-1]
    return np.ascontiguousarray(fp.reshape(H * W, 2 * C))


def _percore_inputs(featp_by_batch, tables, core):
    y_lo, y_hi, xb, hy, ly, wx0, wx1 = tables
    bat, half = divmod(core, 2)
    sl = slice(half * BOX_PER_CORE, (half + 1) * BOX_PER_CORE)
    ylo = y_lo[bat, sl]  # [128, 7]
    xbs = xb[bat, sl]
    # flat pixel index of the 2x2 block base, [128 box, 7 i, 7 j]
    i0 = (ylo[:, :, None] * W + xbs[:, None, :]).astype(np.int32)

    # gather sequence: g = (j*7 + i)*128 + box  (j-major sample order)
    seq = np.transpose(i0, (2, 1, 0)).reshape(NIDX).astype(np.int16)
    wr = seq.reshape(WCOLS, 16).T  # [16, WCOLS]
    idx = np.tile(wr, (8, 1))      # replicate across the 8 gpsimd cores

    hys = hy[bat, sl]    # [128, 7] per sample-row i
    lys = ly[bat, sl]
    wts = np.concatenate([hys, lys], axis=1).astype(np.float32)

    # diag stationaries [128, 14, 128] fp16: slot 2*j+xtap = diag(wx_xtap(:, j))
    wd = np.zeros((128, 14, 128), dtype=np.float16)
    pidx = np.arange(128)
    wvals = np.empty((128, 14), dtype=np.float16)
    wvals[:, 0::2] = wx0[bat, sl]
    wvals[:, 1::2] = wx1[bat, sl]
    wd[pidx[:, None], np.arange(14)[None, :], pidx[:, None]] = wvals

    return {
        "feat": featp_by_batch[bat],
        "idx": np.ascontiguousarray(idx),
        "wts": np.ascontiguousarray(wts),
        "wdiag": np.ascontiguousarray(wd.reshape(128, 14 * 128)),
    }


def _reference_numpy(feats, boxes):
    """Generic fallback: straight numpy port of the reference (never used
    with the reference input distribution; kept for safety)."""
    f32 = np.float32
    L = len(feats)
    padded = np.zeros((B, L, H, W, C), dtype=f32)
    for i, f in enumerate(feats):
        padded[:, i, : f.shape[1], : f.shape[2], :] = f
    b = boxes.astype(f32)
    box_h = b[..., 2] - b[..., 0]
    box_w = b[..., 3] - b[..., 1]
    area = np.sqrt(box_h * box_w)
    lev = np.floor(np.log(area / f32(224.0)) / np.log(f32(2.0))) + f32(4.0)
    levels = np.clip(lev.astype(np.int32), 4, 64)
    scale = np.exp2(levels.astype(f32))
    bs = b / scale[..., None]
    bh = box_h / scale
    bw = box_w / scale
    yxhw = np.concatenate([bs[..., 0:2], bh[..., None], bw[..., None]], axis=-1)
    lvl = levels - 4
    strides = np.exp2(lvl.astype(f32))
    bnd_h = H / strides - f32(1.0)
    bnd_w = W / strides - f32(1.0)
    by = bnd_w[..., None]  # faithful swap from the reference
    bx = bnd_h[..., None]
    box_y = yxhw[..., 0] - f32(0.5)
    box_x = yxhw[..., 1] - f32(0.5)
    offs = (np.arange(OUT, dtype=f32) + f32(0.5)) / f32(OUT)
    gy = box_y[..., None] + offs * yxhw[..., 2:3]
    gx = box_x[..., None] + offs * yxhw[..., 3:4]
    y0 = np.maximum(f32(0.0), np.floor(gy))
    x0 = np.maximum(f32(0.0), np.floor(gx))
    y01 = np.stack([np.minimum(y0, by), np.minimum(y0 + 1, by)], axis=3).reshape(
        B, N, 2 * OUT
    )
    x01 = np.stack([np.minimum(x0, bx), np.minimum(x0 + 1, bx)], axis=3).reshape(
        B, N, 2 * OUT
    )
    yi = y01.astype(np.int32)
    xi = x01.astype(np.int32)
    bi = np.arange(B)[:, None, None, None]
    li = np.clip(lvl, 0, L - 1)[:, :, None, None]
    gathered = padded[bi, li, yi[:, :, :, None], xi[:, :, None, :]]
    ly = gy - y0
    lx = gx - x0
    hy = 1.0 - ly
    hx = 1.0 - lx
    ky = np.stack([hy, ly], axis=3).reshape(B, N, 2 * OUT, 1)
    kx = np.stack([hx, lx], axis=3).reshape(B, N, 1, 2 * OUT)
    kern = (ky * kx * 4.0).astype(f32)
    weighted = gathered * kern[..., None]
    out = weighted.reshape(B, N, OUT, 2, OUT, 2, C).mean(axis=(3, 5))
    return out.astype(f32)


_TRACE_TMPDIR = None


def _run(in_maps, trace=False):
    from concourse.bass_utils import run_bass_kernel_spmd

    nc = _build_nc()
    kw = {}
    if trace and _TRACE_TMPDIR:
        kw["tmpdir"] = _TRACE_TMPDIR
    return run_bass_kernel_spmd(nc, in_maps, list(range(NCORES)), trace=trace, **kw)


def _kernel_impl(inputs, trace=False):
    feats = [np.asarray(inputs[f"feat{i}"], dtype=np.float32) for i in range(5)]
    boxes = np.asarray(inputs["boxes"], dtype=np.float32)
    tables = _host_tables(boxes)
    if tables is None:
        return _reference_numpy(feats, boxes), None
    featp = [_feat_pairs(feats[0][b]) for b in range(B)]
    in_maps = [_percore_inputs(featp, tables, c) for c in range(NCORES)]
    res = _run(in_maps, trace=trace)
    full = np.empty((B, N, OUT, OUT, C), dtype=np.float32)
    for core in range(NCORES):
        bat, half = divmod(core, 2)
        # device sample order is (i, j) already
        o = res.results[core]["out"].astype(np.float32).reshape(
            BOX_PER_CORE, OUT, OUT, C
        )
        full[bat, half * BOX_PER_CORE : (half + 1) * BOX_PER_CORE] = o
    return full, res


def kernel(**inputs):
    out, _ = _kernel_impl(inputs)
    return out


def kernel_profiled(**inputs):
    """Like kernel() but with trace=True; returns (output, BassKernelResults)."""
    return _kernel_impl(inputs, trace=True)
